# revision 30
# baseline (speedup 1.0000x reference)
"""Trainium2 Bass kernel for nn_BAAMamba (VMamba-style 4-direction Mamba classifier).

Sharding: pure data-parallel over batch — 8 cores x 1 image, each core runs the
full model on its image. No collectives.

v1 perf rework vs baseline:
  - bf16 weights for in/x/dt/out projections and perm matrices (PE 2x, DMA 1/2)
  - bf16 b/h cubes; a-cube stays f32 so the DVE scan runs at full (mixed) rate
  - direct Silu activations (kills 6 DVE muls/mixer + table churn)
  - a-cube split: 1 e-tile via 16 ACT exp(scale=A) calls, 2 e-tiles via Pool
    broadcast-mul (delta x A) + one big flat ACT exp  (engine balance)
  - g-mul in-place on h + tree-fold reduce instead of strided tensor_reduce
  - out_proj PSUM folded straight into residual (no hid tile, no ACT copies)
  - B/C broadcast fused into one bf16 DRAM round trip
  - ACT emission ordered so only 2 activation-table loads per mixer
"""

import os
import sys

import numpy as np

sys.path.insert(0, "/opt/trn_rl_repo")

import concourse.bass as bass  # noqa: E402
import concourse.bacc as bacc  # noqa: E402
import concourse.tile as tile  # noqa: E402
from concourse import mybir  # noqa: E402

F32 = mybir.dt.float32
BF16 = mybir.dt.bfloat16
AF = mybir.ActivationFunctionType
ALU = mybir.AluOpType

B = 8
IMG = 224
PATCH = 16
D = 192
DEPTH = 8
H = IMG // PATCH
W = H
L = H * W                      # 196
D_IN = 384
N_ST = 16                      # D_STATE
DT_R = 12
NCLS = 1000
EPS = 1e-5

TS = [(0, 128), (128, L - 128)]          # t tiles (offset, size)
KD = [(0, 128), (128, D - 128)]          # d=192 contraction tiles
NE = D_IN // 128                         # 3 e-tiles
POOL_DA = (0, 1, 2)                      # e-tiles whose delta*A runs on Pool


def build_nc(ndirs=4, ndepth=DEPTH):
    nc = bacc.Bacc("TRN2")

    # ---- DRAM I/O ----
    xcol = nc.dram_tensor("xcol", (768, L), BF16, kind="ExternalInput")
    pwT = nc.dram_tensor("pwT", (768, D), BF16, kind="ExternalInput")
    pb = nc.dram_tensor("pb", (D,), F32, kind="ExternalInput")
    pe_g = nc.dram_tensor("pe_g", (D,), F32, kind="ExternalInput")
    pe_b = nc.dram_tensor("pe_b", (D,), F32, kind="ExternalInput")
    lnw = nc.dram_tensor("lnw", (4, DEPTH, D), F32, kind="ExternalInput")
    lnb = nc.dram_tensor("lnb", (4, DEPTH, D), F32, kind="ExternalInput")
    WinT = nc.dram_tensor("WinT", (4, DEPTH, D, 2 * D_IN), BF16, kind="ExternalInput")
    convwP = nc.dram_tensor("convwP", (4, DEPTH, 128, NE * 4), F32, kind="ExternalInput")
    convbP = nc.dram_tensor("convbP", (4, DEPTH, 128, NE), F32, kind="ExternalInput")
    WxT = nc.dram_tensor("WxT", (4, DEPTH, D_IN, DT_R + 2 * N_ST), BF16, kind="ExternalInput")
    dtwT = nc.dram_tensor("dtwT", (4, DEPTH, DT_R, D_IN), BF16, kind="ExternalInput")
    dtbP = nc.dram_tensor("dtbP", (4, DEPTH, 128, NE), F32, kind="ExternalInput")
    AnegP = nc.dram_tensor("AnegP", (4, DEPTH, 128, NE * N_ST), BF16, kind="ExternalInput")
    DpP = nc.dram_tensor("DpP", (4, DEPTH, 128, NE), F32, kind="ExternalInput")
    WoT = nc.dram_tensor("WoT", (4, DEPTH, D_IN, D), BF16, kind="ExternalInput")
    onw = nc.dram_tensor("onw", (D,), F32, kind="ExternalInput")
    onb = nc.dram_tensor("onb", (D,), F32, kind="ExternalInput")
    hlw = nc.dram_tensor("hlw", (D,), F32, kind="ExternalInput")
    hlb = nc.dram_tensor("hlb", (D,), F32, kind="ExternalInput")
    hwT = nc.dram_tensor("hwT", (D, NCLS), BF16, kind="ExternalInput")
    hb = nc.dram_tensor("hb", (NCLS,), F32, kind="ExternalInput")
    perm = nc.dram_tensor("perm", (4, L, L), BF16, kind="ExternalInput")
    permI = nc.dram_tensor("permI", (4, L, L), BF16, kind="ExternalInput")
    logits = nc.dram_tensor("logits", (1, NCLS), F32, kind="ExternalOutput")

    with tile.TileContext(nc) as tc:
        _emit(nc, tc, locals(), ndirs, ndepth)
    nc.compile()
    if not nc.is_finalized():
        nc.finalize()
    return nc


def _emit(nc, tc, t_, ndirs, ndepth):
    from contextlib import ExitStack

    with ExitStack() as ctx:
        consts = ctx.enter_context(tc.tile_pool(name="consts", bufs=1))
        wpool = ctx.enter_context(tc.tile_pool(name="wpool", bufs=2))
        state = ctx.enter_context(tc.tile_pool(name="state", bufs=1))
        apool = ctx.enter_context(tc.tile_pool(name="apool", bufs=2))
        small = ctx.enter_context(tc.tile_pool(name="small", bufs=3))
        cpool = ctx.enter_context(tc.tile_pool(name="cpool", bufs=2))
        ps1 = ctx.enter_context(tc.tile_pool(name="ps1", bufs=4, space="PSUM"))
        dpool = ctx.enter_context(tc.tile_pool(name="dpool", bufs=2, space="DRAM"))

        # ---- constants ----
        from concourse.masks import make_identity

        ident = consts.tile([128, 128], F32)
        make_identity(nc, ident[:])

        P_sb = []
        PI_sb = []
        for di in range(4):
            p = consts.tile([128, 2, L], BF16, tag=f"P{di}")
            pi = consts.tile([128, 2, L], BF16, tag=f"PI{di}")
            for kt, (koff, ksz) in enumerate(TS):
                nc.sync.dma_start(p[:ksz, kt, :], t_["perm"][di, koff:koff + ksz, :])
                nc.sync.dma_start(pi[:ksz, kt, :], t_["permI"][di, koff:koff + ksz, :])
            P_sb.append(p)
            PI_sb.append(pi)

        # replicated [128, D] per-free-dim vectors
        def rep_vec(name):
            v = consts.tile([128, D], F32, tag=f"rep_{name}")
            nc.sync.dma_start(v[:], t_[name][:].unsqueeze(0).broadcast_to((128, D)))
            return v

        pb_r = rep_vec("pb")
        peg_r = rep_vec("pe_g")
        peb_r = rep_vec("pe_b")
        onw_r = rep_vec("onw")
        onb_r = rep_vec("onb")
        hlw_r = rep_vec("hlw")
        hlb_r = rep_vec("hlb")

        hb_sb = consts.tile([1, NCLS], F32)
        nc.sync.dma_start(hb_sb[:], t_["hb"][:].unsqueeze(0))
        hwT_sb = consts.tile([128, 2, NCLS], BF16)
        for kd, (doff, dsz) in enumerate(KD):
            nc.sync.dma_start(hwT_sb[:dsz, kd, :], t_["hwT"][doff:doff + dsz, :])

        pwT_sb = consts.tile([128, 6, D], BF16)
        col_sb = consts.tile([128, 6, L], BF16)
        for kt in range(6):
            nc.sync.dma_start(pwT_sb[:, kt, :], t_["pwT"][kt * 128:(kt + 1) * 128, :])
            nc.sync.dma_start(col_sb[:, kt, :], t_["xcol"][kt * 128:(kt + 1) * 128, :])

        onescol = consts.tile([128, 1], F32)
        nc.vector.memset(onescol[:], 1.0 / L)
        eps_t = consts.tile([128, 1], F32)
        nc.vector.memset(eps_t[:], EPS)
        t_["eps_t"] = eps_t

        # ---- helpers ----
        def emit_ln(dst_xhat, src, tag):
            """src, dst: [128, 2, D] t-tiled activations; writes xhat (no affine)."""
            for tt, (toff, tsz) in enumerate(TS):
                st6 = small.tile([128, 6], F32, tag="bn6")
                mv = small.tile([128, 2], F32, tag="bn2")
                nc.vector.bn_stats(st6[:tsz], src[:tsz, tt, :])
                nc.vector.bn_aggr(mv[:tsz], st6[:tsz])
                lnv = small.tile([128, 1], F32, tag="lnv")
                rstd = small.tile([128, 1], F32, tag="rstd")
                nc.scalar.activation(lnv[:tsz], mv[:tsz, 1:2], AF.Ln, bias=eps_t[:tsz, :])
                nc.scalar.activation(rstd[:tsz], lnv[:tsz], AF.Exp, scale=-0.5)
                nc.vector.tensor_scalar(
                    out=dst_xhat[:tsz, tt, :], in0=src[:tsz, tt, :],
                    scalar1=mv[:tsz, 0:1], scalar2=rstd[:tsz, 0:1],
                    op0=ALU.subtract, op1=ALU.mult)

        # ---- patch embed ----
        feat_f32 = state.tile([128, 2, D], F32, tag="feat_f32")
        for tt, (toff, tsz) in enumerate(TS):
            ps = ps1.tile([128, D], F32, tag="sps")
            for kt in range(6):
                nc.tensor.matmul(ps[:tsz, :], col_sb[:, kt, toff:toff + tsz],
                                 pwT_sb[:, kt, :], start=(kt == 0), stop=(kt == 5))
            nc.vector.tensor_add(feat_f32[:tsz, tt, :], ps[:tsz, :], pb_r[:tsz, :])
        xh32 = state.tile([128, 2, D], F32, tag="xh32")
        emit_ln(xh32, feat_f32, "pe")
        # feat = xhat * pe_g + pe_b  (keep f32 then make bf16 copies for matmuls)
        for tt, (toff, tsz) in enumerate(TS):
            nc.vector.tensor_mul(feat_f32[:tsz, tt, :], xh32[:tsz, tt, :], peg_r[:tsz, :])
            nc.vector.tensor_add(feat_f32[:tsz, tt, :], feat_f32[:tsz, tt, :], peb_r[:tsz, :])
        # shared depth-0 block-LN xhat of feat
        emit_ln(xh32, feat_f32, "blk0")
        feat16 = state.tile([128, 2, D], BF16, tag="feat16")
        xhat0 = state.tile([128, 2, D], BF16, tag="xhat0")
        for tt, (toff, tsz) in enumerate(TS):
            nc.scalar.copy(feat16[:tsz, tt, :], feat_f32[:tsz, tt, :])
            nc.scalar.copy(xhat0[:tsz, tt, :], xh32[:tsz, tt, :])

        # ---- per-direction residual state (f32) ----
        res_t = [state.tile([128, 2, D], F32, tag=f"res{di}", name=f"res{di}") for di in range(ndirs)]

        for di in range(ndirs):
            for tt, (toff, tsz) in enumerate(TS):
                ps = ps1.tile([128, D], F32, tag="sps")
                for kt, (koff, ksz) in enumerate(TS):
                    nc.tensor.matmul(ps[:tsz, :], P_sb[di][:ksz, kt, toff:toff + tsz],
                                     feat16[:ksz, kt, :], start=(kt == 0), stop=(kt == 1))
                nc.scalar.copy(res_t[di][:tsz, tt, :], ps[:tsz, :])

        # ---- mixer blocks, software-pipelined: emit p1(k+1) before p2(k) so
        # ACT/Pool/PE always have ready work while DVE runs mixer k's scans.
        mixers = [(dep, di) for dep in range(ndepth) for di in range(ndirs)]
        pools = (wpool, apool, small, cpool, ps1, dpool, state)
        pending = None
        for dep, di in mixers:
            st = _mixer_p1(nc, tc, t_, di, dep, res_t[di],
                           xhat0 if dep == 0 else None, P_sb[di], ident, pools)
            if pending is not None:
                _mixer_p2(nc, tc, t_, pools, pending)
            pending = st
        _mixer_p2(nc, tc, t_, pools, pending)

        # ---- CrossMerge (res_t already includes the last hidden) ----
        res16 = state.tile([128, 2, D, 4], BF16, tag="res16")
        for di in range(ndirs):
            for tt, (toff, tsz) in enumerate(TS):
                nc.scalar.copy(res16[:tsz, tt, :, di], res_t[di][:tsz, tt, :])
        merged = state.tile([128, 2, D], F32, tag="merged")
        for tt, (toff, tsz) in enumerate(TS):
            ps = ps1.tile([128, D], F32, tag="sps")
            nmm = ndirs * 2
            i = 0
            for di in range(ndirs):
                for kt, (koff, ksz) in enumerate(TS):
                    nc.tensor.matmul(ps[:tsz, :], PI_sb[di][:ksz, kt, toff:toff + tsz],
                                     res16[:ksz, kt, :, di], start=(i == 0), stop=(i == nmm - 1))
                    i += 1
            nc.scalar.copy(merged[:tsz, tt, :], ps[:tsz, :])

        # ---- out_norm LN + head LN ----
        xh = state.tile([128, 2, D], F32, tag="xh_final")
        emit_ln(xh, merged, "on")
        for tt, (toff, tsz) in enumerate(TS):
            nc.vector.tensor_mul(merged[:tsz, tt, :], xh[:tsz, tt, :], onw_r[:tsz, :])
            nc.vector.tensor_add(merged[:tsz, tt, :], merged[:tsz, tt, :], onb_r[:tsz, :])
        emit_ln(xh, merged, "hl")
        for tt, (toff, tsz) in enumerate(TS):
            nc.vector.tensor_mul(merged[:tsz, tt, :], xh[:tsz, tt, :], hlw_r[:tsz, :])
            nc.vector.tensor_add(merged[:tsz, tt, :], merged[:tsz, tt, :], hlb_r[:tsz, :])

        # ---- mean pool (x 1/L via ones value) ----
        psp = ps1.tile([1, D], F32, tag="sps")
        for kt, (koff, ksz) in enumerate(TS):
            nc.tensor.matmul(psp[:, :], onescol[:ksz, :], merged[:ksz, kt, :],
                             start=(kt == 0), stop=(kt == 1))
        pooled = small.tile([1, D], F32, tag="pooled")
        nc.scalar.copy(pooled[:], psp[:])
        # transpose pooled [1, 192] -> [192, 1]
        pooledT = small.tile([128, 2, 1], BF16, tag="pooledT")
        for kd, (doff, dsz) in enumerate(KD):
            pst = ps1.tile([128, 1], F32, tag="sps")
            nc.tensor.transpose(pst[:dsz, :], pooled[:, doff:doff + dsz], ident[:1, :1])
            nc.scalar.copy(pooledT[:dsz, kd, :], pst[:dsz, :])

        # ---- head ----
        log_sb = small.tile([1, NCLS], F32, tag="logsb")
        for half in range(2):
            psh = ps1.tile([1, 500], F32, tag="sps")
            for kd, (doff, dsz) in enumerate(KD):
                nc.tensor.matmul(psh[:, :], pooledT[:dsz, kd, :],
                                 hwT_sb[:dsz, kd, half * 500:(half + 1) * 500],
                                 start=(kd == 0), stop=(kd == 1))
            nc.vector.tensor_add(log_sb[:, half * 500:(half + 1) * 500], psh[:, :],
                                 hb_sb[:, half * 500:(half + 1) * 500])
        nc.sync.dma_start(t_["logits"][:], log_sb[:])


def _mixer_p1(nc, tc, t_, di, dep, res, xhat0, P_di, ident, pools):
    wpool, apool, small, cpool, ps1, dpool, state = pools
    # ---- stream weights ----
    WinT_sb = wpool.tile([128, 2, 2 * D_IN], BF16, tag="WinT")
    for kd, (doff, dsz) in enumerate(KD):
        nc.sync.dma_start(WinT_sb[:dsz, kd, :], t_["WinT"][di, dep, doff:doff + dsz, :])
    WxT_sb = wpool.tile([128, NE, 44], BF16, tag="WxT")
    WoT_sb = wpool.tile([128, NE, D], BF16, tag="WoT")
    dtwT_sb = wpool.tile([DT_R, NE, 128], BF16, tag="dtwT")
    for ke in range(NE):
        nc.sync.dma_start(WxT_sb[:, ke, :], t_["WxT"][di, dep, ke * 128:(ke + 1) * 128, :])
        nc.sync.dma_start(WoT_sb[:, ke, :], t_["WoT"][di, dep, ke * 128:(ke + 1) * 128, :])
        nc.sync.dma_start(dtwT_sb[:, ke, :], t_["dtwT"][di, dep, :, ke * 128:(ke + 1) * 128])
    # small per-channel params, host-prepacked as [128, NE*...] contiguous rows
    Aneg_sb = wpool.tile([128, NE, N_ST], BF16, tag="Aneg")
    nc.sync.dma_start(Aneg_sb[:], t_["AnegP"][di, dep].rearrange("p (a n) -> p a n", a=NE))
    convw_sb = wpool.tile([128, NE, 4], F32, tag="convw")
    nc.sync.dma_start(convw_sb[:], t_["convwP"][di, dep].rearrange("p (a n) -> p a n", a=NE))
    convb_sb = wpool.tile([128, NE], F32, tag="convb")
    nc.sync.dma_start(convb_sb[:], t_["convbP"][di, dep])
    dtb_sb = wpool.tile([128, NE], F32, tag="dtb")
    nc.sync.dma_start(dtb_sb[:], t_["dtbP"][di, dep])
    Dp_sb = wpool.tile([128, NE], F32, tag="Dp")
    nc.sync.dma_start(Dp_sb[:], t_["DpP"][di, dep])
    lnw_sb = wpool.tile([128, 2], F32, tag="lnw")
    lnb_sb = wpool.tile([128, 2], F32, tag="lnb")
    for kd, (doff, dsz) in enumerate(KD):
        nc.sync.dma_start(lnw_sb[:dsz, kd:kd + 1], t_["lnw"][di, dep, doff:doff + dsz].unsqueeze(1))
        nc.sync.dma_start(lnb_sb[:dsz, kd:kd + 1], t_["lnb"][di, dep, doff:doff + dsz].unsqueeze(1))

    # ---- xlnT [d-part(2), L] bf16 ----
    xlnT = apool.tile([128, 2, L], BF16, tag="xlnT")
    if xhat0 is not None:
        # depth 0: permute shared xhat0 via P matmuls
        for kd, (doff, dsz) in enumerate(KD):
            ps = ps1.tile([128, L], F32, tag="sps")
            for kt, (koff, ksz) in enumerate(TS):
                nc.tensor.matmul(ps[:dsz, :], xhat0[:ksz, kt, doff:doff + dsz],
                                 P_di[:ksz, kt, :], start=(kt == 0), stop=(kt == 1))
            nc.vector.tensor_scalar(
                out=xlnT[:dsz, kd, :], in0=ps[:dsz, :],
                scalar1=lnw_sb[:dsz, kd:kd + 1], scalar2=lnb_sb[:dsz, kd:kd + 1],
                op0=ALU.mult, op1=ALU.add)
    else:
        # LN(res) ; transpose  (res already includes previous hidden)
        xhat = apool.tile([128, 2, D], F32, tag="xhat", bufs=1)
        mvs, lnvs, rstds = [], [], []
        for tt, (toff, tsz) in enumerate(TS):
            st6 = small.tile([128, 6], F32, tag="bn6")
            mv = small.tile([128, 2], F32, tag="bn2")
            nc.vector.bn_stats(st6[:tsz], res[:tsz, tt, :])
            nc.vector.bn_aggr(mv[:tsz], st6[:tsz])
            mvs.append(mv)
            lnv = small.tile([128, 1], F32, tag="lnv")
            lnvs.append(lnv)
            rstd = small.tile([128, 1], F32, tag="rstd")
            rstds.append(rstd)
        for tt, (toff, tsz) in enumerate(TS):
            nc.scalar.activation(lnvs[tt][:tsz], mvs[tt][:tsz, 1:2], AF.Ln,
                                 bias=t_["eps_t"][:tsz, :])
        for tt, (toff, tsz) in enumerate(TS):
            nc.scalar.activation(rstds[tt][:tsz], lnvs[tt][:tsz], AF.Exp, scale=-0.5)
        for tt, (toff, tsz) in enumerate(TS):
            nc.vector.tensor_scalar(
                out=xhat[:tsz, tt, :], in0=res[:tsz, tt, :],
                scalar1=mvs[tt][:tsz, 0:1], scalar2=rstds[tt][:tsz, 0:1],
                op0=ALU.subtract, op1=ALU.mult)
        for kd, (doff, dsz) in enumerate(KD):
            ps = ps1.tile([128, L], F32, tag="sps")
            for tt, (toff, tsz) in enumerate(TS):
                nc.tensor.transpose(ps[:dsz, toff:toff + tsz],
                                    xhat[:tsz, tt, doff:doff + dsz], ident[:tsz, :tsz])
            nc.vector.tensor_scalar(
                out=xlnT[:dsz, kd, :], in0=ps[:dsz, :],
                scalar1=lnw_sb[:dsz, kd:kd + 1], scalar2=lnb_sb[:dsz, kd:kd + 1],
                op0=ALU.mult, op1=ALU.add)

    # ---- in_proj + causal conv + silu (u path), silu (z path) ----
    uraw = apool.tile([128, NE, L], BF16, tag="uraw", bufs=1)
    acc = apool.tile([128, NE, L], BF16, tag="acc", bufs=1)
    u2 = apool.tile([128, NE, L], BF16, tag="u2")
    sz = apool.tile([128, NE, L], BF16, tag="sz", bufs=2)
    for ec in range(6):
        ps = ps1.tile([128, L], F32, tag="sps")
        for kd, (doff, dsz) in enumerate(KD):
            nc.tensor.matmul(ps[:, :], WinT_sb[:dsz, kd, ec * 128:(ec + 1) * 128],
                             xlnT[:dsz, kd, :], start=(kd == 0), stop=(kd == 1))
        if ec < NE:
            nc.scalar.copy(uraw[:, ec, :], ps[:, :])
            nc.vector.tensor_scalar(out=acc[:, ec, :], in0=uraw[:, ec, :],
                                    scalar1=convw_sb[:, ec, 3:4],
                                    scalar2=convb_sb[:, ec:ec + 1],
                                    op0=ALU.mult, op1=ALU.add)
            for k in range(1, 4):
                nc.vector.affine_then_add(
                    out=acc[:, ec, k:L], in0=uraw[:, ec, 0:L - k],
                    in1=acc[:, ec, k:L],
                    scale=convw_sb[:, ec, 3 - k:4 - k], bias=0.0)
            nc.scalar.activation(u2[:, ec, :], acc[:, ec, :], AF.Silu)
        else:
            nc.scalar.activation(sz[:, ec - NE, :], ps[:, :], AF.Silu)

    # ---- x_proj (dt / BC split) ----
    dtm = apool.tile([DT_R, L], BF16, tag="dtm", bufs=1)
    BC_sb = apool.tile([2 * N_ST, L], BF16, tag="BC", bufs=1)
    for si, (soff, ssz, dst) in enumerate(
            [(0, DT_R, None), (DT_R, 2 * N_ST, None)]):
        psx = ps1.tile([2 * N_ST, L], F32, tag="spsx", bufs=2)
        for ke in range(NE):
            nc.tensor.matmul(psx[:ssz, :], WxT_sb[:, ke, soff:soff + ssz],
                             u2[:, ke, :], start=(ke == 0), stop=(ke == NE - 1))
        nc.scalar.copy((dtm if si == 0 else BC_sb)[:ssz, :], psx[:ssz, :])

    # ---- B_r / C_r broadcast via one bf16 DRAM round-trip ----
    bc_dram = dpool.tile([1, 2 * N_ST * L], BF16, tag="bc_dram")
    nc.sync.dma_start(bc_dram[:].rearrange("a (n t) -> (a n) t", t=L), BC_sb[:, :])
    BC_r = cpool.tile([128, 2 * N_ST, L], BF16, tag="BCrep", bufs=2)
    nc.sync.dma_start(BC_r[:].rearrange("p n t -> p (n t)"),
                      bc_dram[:].broadcast_to((128, 2 * N_ST * L)))

    # ---- delta (softplus; exp intermediate kept in PSUM; bf16 out) ----
    delta16 = apool.tile([128, NE, L], BF16, tag="delta16", bufs=2)
    for ec in range(NE):
        psd = ps1.tile([128, L], F32, tag="sps")
        nc.tensor.matmul(psd[:, :], dtwT_sb[:, ec, :], dtm[:, :],
                         start=True, stop=True)
        spe = ps1.tile([128, L], F32, tag="spe", bufs=2)
        nc.scalar.activation(spe[:, :], psd[:, :], AF.Exp, bias=dtb_sb[:, ec:ec + 1])
        nc.scalar.activation(delta16[:, ec, :], spe[:, :], AF.Ln, bias=1.0)

    # ---- a-cube (f32): Pool builds delta*A, ACT exps it; ec0 double-buffered
    a_et = []
    for ec in range(NE):
        a1 = cpool.tile([128, N_ST, L], BF16, tag=f"acube{ec}",
                        bufs=(2 if ec == 0 else 1))
        da = cpool.tile([128, N_ST, L], BF16, tag="da", bufs=2)
        d_b = delta16[:, ec, :].unsqueeze(1).broadcast_to((128, N_ST, L))
        A_b = Aneg_sb[:, ec, :].unsqueeze(2).broadcast_to((128, N_ST, L))
        nc.gpsimd.tensor_mul(da[:], d_b, A_b)
        nc.scalar.activation(a1[:].rearrange("p n t -> p (n t)"),
                             da[:].rearrange("p n t -> p (n t)"), AF.Exp)
        nc.gpsimd.memset(a1[:, :, 0:1], 0.0)
        a_et.append(a1)

    return dict(res=res, u2=u2, sz=sz, delta16=delta16, BC_r=BC_r, a_et=a_et,
                WoT_sb=WoT_sb, Dp_sb=Dp_sb)


def _mixer_p2(nc, tc, t_, pools, st):
    wpool, apool, small, cpool, ps1, dpool, state = pools
    res, u2, sz = st["res"], st["u2"], st["sz"]
    BC_r, a_et = st["BC_r"], st["a_et"]
    WoT_sb, Dp_sb = st["WoT_sb"], st["Dp_sb"]

    # v = delta * u2, in place on delta16
    v = st["delta16"]
    nc.vector.tensor_mul(v[:].rearrange("p n t -> p (n t)"),
                         v[:].rearrange("p n t -> p (n t)"),
                         u2[:].rearrange("p n t -> p (n t)"))

    b_sb = cpool.tile([128, NE, N_ST, L], BF16, tag="bcube", bufs=1)
    h_sb = cpool.tile([128, NE, N_ST, L], BF16, tag="hcube", bufs=1)
    y_sb = apool.tile([128, NE, L], BF16, tag="ysb", bufs=1)
    with nc.allow_low_precision(reason="bf16 tree reduce, validated offline"):
        for ec in range(NE):
            v_b = v[:, ec, :].unsqueeze(1).broadcast_to((128, N_ST, L))
            nc.vector.tensor_mul(b_sb[:, ec, :, :], BC_r[:, 0:N_ST, :], v_b)
            nc.vector.tensor_tensor_scan(
                out=h_sb[:, ec].rearrange("p n t -> p (n t)"),
                data0=a_et[ec][:].rearrange("p n t -> p (n t)"),
                data1=b_sb[:, ec].rearrange("p n t -> p (n t)"),
                initial=0.0, op0=ALU.mult, op1=ALU.add)
            # y = sum_n h*C via in-place g-mul + tree fold (bf16)
            hh = h_sb[:, ec]
            g8 = cpool.tile([128, 8, L], BF16, tag="g8", bufs=2)
            nc.vector.tensor_mul(hh[:, 0:8, :], hh[:, 0:8, :], BC_r[:, N_ST:N_ST + 8, :])
            nc.vector.tensor_mul(hh[:, 8:16, :], hh[:, 8:16, :], BC_r[:, N_ST + 8:2 * N_ST, :])
            nc.vector.tensor_add(g8[:], hh[:, 0:8, :], hh[:, 8:16, :])
            nc.vector.tensor_add(g8[:, 0:4, :], g8[:, 0:4, :], g8[:, 4:8, :])
            nc.vector.tensor_add(g8[:, 0:2, :], g8[:, 0:2, :], g8[:, 2:4, :])
            nc.vector.tensor_add(y_sb[:, ec, :], g8[:, 0, :], g8[:, 1, :])

        # ---- y2 = y + u2*Dp ; y3 = y2 * silu(z) ----
        for ec in range(NE):
            nc.vector.affine_then_add(out=y_sb[:, ec, :], in0=u2[:, ec, :],
                                      in1=y_sb[:, ec, :],
                                      scale=Dp_sb[:, ec:ec + 1], bias=0.0)
        nc.vector.tensor_mul(y_sb[:].rearrange("p n t -> p (n t)"),
                             y_sb[:].rearrange("p n t -> p (n t)"),
                             sz[:].rearrange("p n t -> p (n t)"))

    # ---- out_proj -> residual (PSUM folded straight into res) ----
    for tt, (toff, tsz) in enumerate(TS):
        pso = ps1.tile([128, D], F32, tag="sps")
        for ke in range(NE):
            nc.tensor.matmul(pso[:tsz, :], y_sb[:, ke, toff:toff + tsz], WoT_sb[:, ke, :],
                             start=(ke == 0), stop=(ke == NE - 1))
        nc.vector.tensor_add(res[:tsz, tt, :], res[:tsz, tt, :], pso[:tsz, :])


# ============================== host side ==============================

_NC_CACHE = {}


def _get_nc():
    if "nc" not in _NC_CACHE:
        _NC_CACHE["nc"] = build_nc()
    return _NC_CACHE["nc"]


def _perm_matrices():
    idx = np.arange(L).reshape(H, W)
    perm0 = idx.reshape(-1)
    perm1 = idx.T.reshape(-1)
    perms = [perm0, perm1, perm0[::-1].copy(), perm1[::-1].copy()]
    P = np.zeros((4, L, L), np.float32)
    PI = np.zeros((4, L, L), np.float32)
    for di, pm in enumerate(perms):
        P[di, pm, np.arange(L)] = 1.0       # seq[t'] = sum_t P[t,t'] feat[t]
        PI[di] = P[di].T                     # merged[t] = sum_t' PI[t',t] out[t']
    return P, PI


def prep_inputs(inputs):
    """Host-side layout prep. Returns (shared weight map, per-core xcol list)."""
    import ml_dtypes

    BF = ml_dtypes.bfloat16
    g = {k: np.ascontiguousarray(np.asarray(v, dtype=np.float32)) for k, v in inputs.items()}
    P, PI = _perm_matrices()

    def packP(w):  # (4,8,384,...) -> (4,8,128,NE*...), partition-major rows
        rest = w.shape[3:]
        q = w.reshape(4, DEPTH, NE, 128, *rest).transpose(0, 1, 3, 2, *range(4, 4 + len(rest)))
        return np.ascontiguousarray(q.reshape(4, DEPTH, 128, -1))

    shared = dict(
        pwT=np.ascontiguousarray(g["patch_w"].reshape(D, 768).T).astype(BF),
        pb=g["patch_b"], pe_g=g["pe_ln_w"], pe_b=g["pe_ln_b"],
        lnw=g["ln_w"], lnb=g["ln_b"],
        WinT=np.ascontiguousarray(g["in_proj_w"].transpose(0, 1, 3, 2)).astype(BF),
        convwP=packP(g["conv_w"]), convbP=packP(g["conv_b"]),
        WxT=np.ascontiguousarray(g["x_proj_w"].transpose(0, 1, 3, 2)).astype(BF),
        dtwT=np.ascontiguousarray(g["dt_w"].transpose(0, 1, 3, 2)).astype(BF),
        dtbP=packP(g["dt_b"]),
        AnegP=packP(-np.exp(g["A_log"])).astype(BF),
        DpP=packP(g["Dp"]),
        WoT=np.ascontiguousarray(g["out_proj_w"].transpose(0, 1, 3, 2)).astype(BF),
        onw=g["out_norm_w"], onb=g["out_norm_b"],
        hlw=g["head_ln_w"], hlb=g["head_ln_b"],
        hwT=np.ascontiguousarray(g["head_w"].T).astype(BF), hb=g["head_b"],
        perm=P.astype(BF), permI=PI.astype(BF),
    )
    x = g["x"]
    xcols = []
    for b in range(x.shape[0]):
        xb = x[b]                                          # (3, 224, 224)
        c = xb.reshape(3, H, PATCH, W, PATCH)              # (3, i, pi, j, pj)
        col = c.transpose(0, 2, 4, 1, 3).reshape(768, L)   # (c,pi,pj),(i,j)
        xcols.append(np.ascontiguousarray(col).astype(BF))
    return shared, xcols


def kernel(**inputs):
    from concourse.bass_utils import run_bass_kernel_spmd

    nc = _get_nc()
    shared, xcols = prep_inputs(inputs)
    nb = len(xcols)
    in_maps = [dict(shared, xcol=xcols[b]) for b in range(nb)]
    res = run_bass_kernel_spmd(nc, in_maps, core_ids=list(range(nb)))
    out = np.stack([res.results[b]["logits"][0] for b in range(nb)])
    return out.astype(np.float32)
